# revision 2
# baseline (speedup 1.0000x reference)
"""ODE-RNN Trainium2 Bass kernel, v2.

Data-parallel over batch across 8 NeuronCores (136 rows/core), state kept
transposed [DRNN=128 partitions, rows free]. Two row-group chains (G=2) are
software-pipelined with a phase offset (OFF) so PE matmuls of one chain
overlap ACT/DVE links of the other.

Matmuls run with fp16 operands (1 PE cycle/row vs fp32's 4). The recurrence
itself stays fp32 on DVE; fp16 enters only at matmul operands. Per-site
noise control (MODES): 'w' = weight split hi+lo fp16 (host-side, 2x MMs),
'x' = moving-operand split hi+lo fp16 (extra MMs + DVE sub). 'wx' gives
~fp32-quality for that site (3 MMs, still cheaper than one fp32 matmul).

ODE Euler steps telescoped through PSUM accumulation (z1 += (DT*w2@w0)^T a2).
Output projection y = wout^T h1 deferred: h1hi (fp16) streams into an SBUF
history buffer; chunked wout matmuls run interleaved with the scan on two
dedicated PSUM banks, copied out on a rotating engine and DMA'd.

PSUM layout (8 banks, explicit bank padding):
  z1a | z1b | hd | z2 (2 halves seq.) | przpg0 (4 seq. slices) |
  przpg1 (4 seq. slices) | py x2 (output chunks)
"""

import os
import numpy as np
import ml_dtypes

B, S, P, J = 32, 128, 2, 17
DIN, DOUT, DRNN, DHID = 2, 3, 128, 256
N = P * J            # 34
DT = 0.1
K = 4                # Euler steps
NCORES = 8
BL = B // NCORES     # 4 batches per core
R = BL * N           # 136 rows per core
G = 2
RG = R // G          # 68

NSTEPS = int(os.environ.get("ODERNN_STEPS", S))
SR = NSTEPS * R
OFF = int(os.environ.get("ODERNN_OFF", "14"))   # phase stagger of chain B
CH = 512                                         # output chunk columns

DEFAULT_MODES = {
    "z1i": "wx", "z2": "w", "tel": "w", "hd": "w",
    "gi0": "wx", "gh0": "wx", "gi1": "wx", "gh1": "wx", "out": "",
}


def _modes_cfg():
    cfg = dict(DEFAULT_MODES)
    s = os.environ.get("ODERNN_MODES", "")
    for part in s.split(","):
        if ":" in part:
            k, v = part.split(":")
            cfg[k] = v.replace("-", "")
    return cfg


MODES = _modes_cfg()

F16 = np.float16
BF16 = ml_dtypes.bfloat16

WSHAPES = {
    "w0": [DRNN, DHID], "w1a": [128, DHID], "w1b": [128, DHID],
    "w2a": [128, DRNN], "w2b": [128, DRNN],
    "W20a": [128, DHID], "W20b": [128, DHID],
    "wih0": [DIN, 3 * DRNN], "whh0": [DRNN, 3 * DRNN],
    "wih1": [DRNN, 3 * DRNN], "whh1": [DRNN, 3 * DRNN],
    "wout": [DRNN, DOUT],
}
WSITE = {
    "w0": "z1i", "w1a": "z2", "w1b": "z2", "w2a": "hd", "w2b": "hd",
    "W20a": "tel", "W20b": "tel", "wih0": "gi0", "whh0": "gh0",
    "wih1": "gi1", "whh1": "gh1", "wout": "out",
}

_prog_cache = {}


def _build_program():
    import concourse.bass as bass
    import concourse.tile as tile
    import concourse.mybir as mybir
    from concourse import bacc

    dt = mybir.dt
    f32 = dt.float32
    f16 = dt.float16
    bf16 = dt.bfloat16
    Alu = mybir.AluOpType
    Act = mybir.ActivationFunctionType

    split_a1 = "x" in MODES["z2"]

    nc = bacc.Bacc("TRN2", target_bir_lowering=False)

    d_xm = nc.dram_tensor("xm", [DIN, SR], f16, kind="ExternalInput")
    d_xml = nc.dram_tensor("xml", [DIN, SR], f16, kind="ExternalInput")
    d_mbc = nc.dram_tensor("mbc", [DRNN, SR], bf16, kind="ExternalInput")
    d_h0f = nc.dram_tensor("h0f", [DRNN, R], f32, kind="ExternalInput")
    d_h016 = nc.dram_tensor("h016", [DRNN, R], f16, kind="ExternalInput")
    DW = {}
    for nm, shape in WSHAPES.items():
        for suf in ("", "_l"):
            DW[nm + suf] = nc.dram_tensor(nm + suf, list(shape), f16,
                                          kind="ExternalInput")
    DW32 = {}
    for nm in ("W20a", "W20b", "w2a", "w2b"):
        DW32[nm] = nc.dram_tensor(nm + "32", list(WSHAPES[nm]), f32,
                                  kind="ExternalInput")
    d_y = nc.dram_tensor("y", [DOUT, SR], f32, kind="ExternalOutput")

    with tile.TileContext(nc) as tc:
        wp = tc.alloc_tile_pool(name="wconst", bufs=1)
        st = tc.alloc_tile_pool(name="state", bufs=3)
        wk = tc.alloc_tile_pool(name="work", bufs=2)
        pp = tc.alloc_tile_pool(name="psum", bufs=1, space="PSUM")

        def load(pool, dramt, shape, dty, name):
            t = pool.tile(shape, dty, tag=name, name=name)
            nc.sync.dma_start(out=t[:], in_=dramt[:])
            return t

        xm = load(wp, d_xm, [DIN, SR], f16, "xm")
        xml = load(wp, d_xml, [DIN, SR], f16, "xml") \
            if "x" in MODES["gi0"] else None
        mbc = load(wp, d_mbc, [DRNN, SR], bf16, "mbc")
        h0f = load(wp, d_h0f, [DRNN, R], f32, "h0f")
        h016 = load(wp, d_h016, [DRNN, R], f16, "h016")
        sb = {}
        for nm, shape in WSHAPES.items():
            sb[nm] = load(wp, DW[nm], shape, f16, nm)
            if "w" in MODES[WSITE[nm]] or nm == "w0":
                sb[nm + "_l"] = load(wp, DW[nm + "_l"], shape, f16, nm + "_l")
        sb32 = {}
        for nm in ("W20a", "W20b", "w2a", "w2b"):
            sb32[nm] = load(wp, DW32[nm], WSHAPES[nm], f32, nm + "32")

        h1all = wp.tile([DRNN, SR], f16, tag="h1all", name="h1all")

        MM = nc.tensor.matmul
        ACT = nc.scalar.activation
        V = nc.vector
        PL = nc.gpsimd

        hprev = [h0f[:, gi * RG:(gi + 1) * RG] for gi in range(G)]

        def mm_seq(out_ap, pairs, start, stop):
            n = len(pairs)
            for i, (w_ap, x_ap) in enumerate(pairs):
                MM(out_ap, w_ap, x_ap,
                   start=(start and i == 0), stop=(stop and i == n - 1))

        def site_pairs(nm, lo, hi, xs, site):
            mode = MODES[site]
            xhi, xlo = xs
            w_main = sb[nm][:, lo:hi]
            pairs = [(w_main, xhi)]
            if "w" in mode:
                pairs.append((sb[nm + "_l"][:, lo:hi], xhi))
            if "x" in mode and xlo is not None:
                pairs.append((w_main, xlo))
            return pairs

        NP = 28
        state = [dict() for _ in range(G)]
        ydone = [0]   # chunks emitted

        def a_ap(stt, which, half):
            """a1/a2 moving operand (hi, lo) for ODE matmuls."""
            hi = stt[which + "hi"]
            lo = stt.get(which + "lo")
            return (hi[:, half, :], lo[:, half, :] if lo is not None else None)

        def emit(s, gi, p):
            c0 = s * R + gi * RG
            c1 = c0 + RG
            stt = state[gi]

            if p == 0:
                # z1ab spans two PSUM banks (bank-aligned halves) so ONE ACT
                # with a strided 3D AP covers both tanh halves per k. For
                # s>0 the z1 accumulation opened early at p18/p25/p27 of the
                # previous step (h1, -g*h1, +g*n1 parts).
                if s == 0:
                    z1ab = pp.tile([128, 2, RG], f32, tag="z1ab", name="z1ab",
                                   bufs=2, padded_shape=[128, 2, 512])
                    stt["z1ab"] = z1ab
                    xs = (h016[:, gi * RG:(gi + 1) * RG], None)
                    mm_seq(z1ab[:, 0, 0:RG],
                           site_pairs("w0", 0, 128, xs, "z1i"),
                           start=True, stop=False)
                    mm_seq(z1ab[:, 1, 0:RG],
                           site_pairs("w0", 128, 256, xs, "z1i"),
                           start=True, stop=False)
                else:
                    stt["z1ab"] = stt["z1ab_next"]
                stt["z1a"] = stt["z1ab"][:, 0, 0:RG]
                stt["z1b"] = stt["z1ab"][:, 1, 0:RG]
                return

            if 1 <= p <= 16:
                k = (p - 1) // 4
                sub = (p - 1) % 4
                z1a, z1b = stt["z1a"], stt["z1b"]
                if sub == 0:
                    # a1 = tanh(z1) -> f16 (or f32 + split when z2 has 'x')
                    z1ab = stt["z1ab"]
                    if split_a1:
                        a1f = wk.tile([128, 2, RG], f32, tag=f"a1f_{gi}",
                                      name="a1f")
                        ACT(a1f[:], z1ab[:, :, 0:RG], Act.Tanh)
                        a1hi = wk.tile([128, 2, RG], f16, tag=f"a1hi_{gi}",
                                       name="a1hi")
                        V.tensor_copy(a1hi[:], a1f[:])
                        a1lo = wk.tile([128, 2, RG], f16, tag=f"a1lo_{gi}",
                                       name="a1lo")
                        V.tensor_sub(a1lo[:], a1f[:], a1hi[:])
                        stt["a1hi"], stt["a1lo"] = a1hi, a1lo
                    else:
                        a1hi = wk.tile([128, 2, RG], f16, tag=f"a1hi_{gi}",
                                       name="a1hi")
                        ACT(a1hi[:], z1ab[:, :, 0:RG], Act.Tanh)
                        stt["a1hi"] = a1hi
                        stt.pop("a1lo", None)
                elif sub == 1:
                    z2 = pp.tile([128, 2, RG], f32, tag="z2", name="z2",
                                 padded_shape=[128, 2, 256])
                    stt["z2"] = z2
                    xs0, xs1 = a_ap(stt, "a1", 0), a_ap(stt, "a1", 1)
                    pairs0 = site_pairs("w1a", 0, 128, xs0, "z2") + \
                        site_pairs("w1b", 0, 128, xs1, "z2")
                    pairs1 = site_pairs("w1a", 128, 256, xs0, "z2") + \
                        site_pairs("w1b", 128, 256, xs1, "z2")
                    mm_seq(z2[:, 0, :], pairs0, start=True, stop=True)
                    mm_seq(z2[:, 1, :], pairs1, start=True, stop=True)
                elif sub == 2:
                    z2 = stt["z2"]
                    a2hi = wk.tile([128, 2, RG], f16, tag=f"a2hi_{gi}",
                                   name="a2hi")
                    ACT(a2hi[:], z2[:], Act.Tanh)
                    stt["a2hi"] = a2hi
                else:
                    xs0, xs1 = a_ap(stt, "a2", 0), a_ap(stt, "a2", 1)
                    if k < K - 1:
                        pairs0 = site_pairs("W20a", 0, 128, xs0, "tel") + \
                            site_pairs("W20b", 0, 128, xs1, "tel")
                        pairs1 = site_pairs("W20a", 128, 256, xs0, "tel") + \
                            site_pairs("W20b", 128, 256, xs1, "tel")
                        mm_seq(z1a[:], pairs0, start=False, stop=(k == K - 2))
                        mm_seq(z1b[:], pairs1, start=False, stop=(k == K - 2))
                    if k == 0:
                        hd = pp.tile([128, RG], f32, tag="hd", name="hd",
                                     padded_shape=[128, 512])
                        stt["hd"] = hd
                    hd = stt["hd"]
                    pairs = site_pairs("w2a", 0, 128, xs0, "hd") + \
                        site_pairs("w2b", 0, 128, xs1, "hd")
                    mm_seq(hd[:], pairs, start=(k == 0), stop=(k == K - 1))
                return

            if p == 17:
                hd = stt["hd"]
                h1hi = h1all[:, c0:c1]
                V.tensor_add(h1hi, hd[:], hprev[gi])   # f16 out (chain)
                stt["h1hi"] = h1hi
                h1f = st.tile([128, RG], f32, tag=f"h1f_{gi}", name="h1f")
                V.tensor_add(h1f[:], hd[:], hprev[gi])
                stt["h1f"] = h1f
                if "x" in MODES["gh0"] or "x" in MODES["gh1"]:
                    h1lo = wk.tile([128, RG], f16, tag=f"h1lo_{gi}",
                                   name="h1lo")
                    V.tensor_sub(h1lo[:], h1f[:], h1hi)
                    stt["h1lo"] = h1lo
                return

            if p == 18:
                h1hi, h1lo = stt["h1hi"], stt.get("h1lo")
                xsl = (xm[:, c0:c1], xml[:, c0:c1] if xml is not None else None)
                hs = (h1hi, h1lo[:] if h1lo is not None else None)
                pp0 = pp.tile([128, 4, RG], f32, tag="przpg0", name="przpg0",
                              padded_shape=[128, 4, 128])
                pg1 = pp.tile([128, 4, RG], f32, tag="przpg1", name="przpg1",
                              padded_shape=[128, 4, 128])
                stt["pp0"], stt["pg1"] = pp0, pg1
                pr = site_pairs("wih0", 0, 128, xsl, "gi0") + \
                    site_pairs("whh0", 0, 128, hs, "gh0")
                pz = site_pairs("wih0", 128, 256, xsl, "gi0") + \
                    site_pairs("whh0", 128, 256, hs, "gh0")
                mm_seq(pp0[:, 0, :], pr, start=True, stop=True)
                mm_seq(pp0[:, 1, :], pz, start=True, stop=True)
                mm_seq(pp0[:, 2, :], site_pairs("wih0", 256, 384, xsl, "gi0"),
                       start=True, stop=True)
                mm_seq(pp0[:, 3, :], site_pairs("whh0", 256, 384, hs, "gh0"),
                       start=True, stop=True)
                mm_seq(pg1[:, 3, :], site_pairs("whh1", 256, 384, hs, "gh1"),
                       start=True, stop=True)
                if s < NSTEPS - 1:
                    z1n = pp.tile([128, 2, RG], f32, tag="z1ab", name="z1ab",
                                  bufs=2, padded_shape=[128, 2, 512])
                    stt["z1ab_next"] = z1n
                    ph0 = site_pairs("w0", 0, 128, hs, "z1i")
                    ph1 = site_pairs("w0", 128, 256, hs, "z1i")
                    if "x" not in MODES["z1i"] and h1lo is not None:
                        ph0 = ph0 + [(sb["w0"][:, 0:128], h1lo[:])]
                        ph1 = ph1 + [(sb["w0"][:, 128:256], h1lo[:])]
                    mm_seq(z1n[:, 0, 0:RG], ph0, start=True, stop=False)
                    mm_seq(z1n[:, 1, 0:RG], ph1, start=True, stop=False)
                return

            if p == 19:
                rz0 = wk.tile([128, 2, RG], f32, tag=f"rz0_{gi}", name="rz0")
                ACT(rz0[:], stt["pp0"][:, 0:2, :], Act.Sigmoid)
                stt["rz0"] = rz0
                return

            if p == 20:
                rz0, pp0, h1f = stt["rz0"], stt["pp0"], stt["h1f"]
                r0, zz0 = rz0[:, 0, :], rz0[:, 1, :]
                u0 = wk.tile([128, RG], f32, tag=f"u0_{gi}", name="u0")
                PL.tensor_scalar(u0[:], zz0, -1.0, 1.0,
                                 op0=Alu.mult, op1=Alu.add)
                t0 = wk.tile([128, RG], f32, tag=f"t0_{gi}", name="t0")
                PL.tensor_mul(t0[:], zz0, h1f[:])
                s1 = wk.tile([128, RG], f32, tag=f"s1_{gi}", name="s1")
                V.tensor_mul(s1[:], r0, pp0[:, 3, :])
                np0 = wk.tile([128, RG], f32, tag=f"np0_{gi}", name="np0")
                V.tensor_add(np0[:], s1[:], pp0[:, 2, :])
                stt["u0"], stt["t0"], stt["np0"] = u0, t0, np0
                return

            if p == 21:
                n0 = wk.tile([128, RG], f32, tag=f"n0_{gi}", name="n0")
                ACT(n0[:], stt["np0"][:], Act.Tanh)
                stt["n0"] = n0
                return

            if p == 22:
                v0 = wk.tile([128, RG], f32, tag=f"v0_{gi}", name="v0")
                PL.tensor_mul(v0[:], stt["u0"][:], stt["n0"][:])
                h2b = wk.tile([128, RG], f32, tag=f"h2b_{gi}", name="h2b")
                PL.tensor_add(h2b[:], v0[:], stt["t0"][:])
                stt["h2b"] = h2b
                h2bhi = wk.tile([128, RG], f16, tag=f"h2bhi_{gi}",
                                name="h2bhi")
                V.tensor_add(h2bhi[:], v0[:], stt["t0"][:])
                stt["h2bhi"] = h2bhi
                if "x" in MODES["gi1"]:
                    h2blo = wk.tile([128, RG], f16, tag=f"h2blo_{gi}",
                                    name="h2blo")
                    V.tensor_sub(h2blo[:], h2b[:], h2bhi[:])
                    stt["h2blo"] = h2blo
                # y projection: reuse przpg0's bank (its reads are done)
                py = pp.tile([DOUT, RG], f32, tag="przpg0", name="py")
                mm_seq(py[:], site_pairs("wout", 0, DOUT,
                                         (stt["h1hi"], None), "out"),
                       start=True, stop=True)
                ysl = wk.tile([DOUT, RG], f32, tag=f"ysl_{gi}", name="ysl")
                V.tensor_copy(ysl[:], py[:])
                nc.sync.dma_start(out=d_y[:, c0:c1], in_=ysl[:])
                return

            if p == 23:
                pg1 = stt["pg1"]
                h1hi, h1lo = stt["h1hi"], stt.get("h1lo")
                hs = (h1hi, h1lo[:] if h1lo is not None else None)
                hs2 = (stt["h2bhi"][:], stt["h2blo"][:]
                       if "x" in MODES["gi1"] else None)
                pr = site_pairs("whh1", 0, 128, hs, "gh1") + \
                    site_pairs("wih1", 0, 128, hs2, "gi1")
                pz = site_pairs("whh1", 128, 256, hs, "gh1") + \
                    site_pairs("wih1", 128, 256, hs2, "gi1")
                mm_seq(pg1[:, 0, :], pr, start=True, stop=True)
                mm_seq(pg1[:, 1, :], pz, start=True, stop=True)
                mm_seq(pg1[:, 2, :], site_pairs("wih1", 256, 384, hs2, "gi1"),
                       start=True, stop=True)
                return

            if p == 24:
                rz1 = wk.tile([128, 2, RG], f32, tag=f"rz1_{gi}", name="rz1")
                ACT(rz1[:], stt["pg1"][:, 0:2, :], Act.Sigmoid)
                stt["rz1"] = rz1
                return

            if p == 25:
                rz1, pg1, h1f = stt["rz1"], stt["pg1"], stt["h1f"]
                r1, zz1 = rz1[:, 0, :], rz1[:, 1, :]
                msl = mbc[:, c0:c1]
                u1 = wk.tile([128, RG], f32, tag=f"u1_{gi}", name="u1")
                PL.tensor_scalar(u1[:], zz1, -1.0, 1.0,
                                 op0=Alu.mult, op1=Alu.add)
                g = wk.tile([128, RG], f32, tag=f"g_{gi}", name="g")
                V.tensor_mul(g[:], u1[:], msl)
                tg = wk.tile([128, RG], f32, tag=f"tg_{gi}", name="tg")
                PL.tensor_mul(tg[:], g[:], h1f[:])
                hm = wk.tile([128, RG], f32, tag=f"hm_{gi}", name="hm")
                PL.tensor_sub(hm[:], h1f[:], tg[:])
                s2 = wk.tile([128, RG], f32, tag=f"s2_{gi}", name="s2")
                V.tensor_mul(s2[:], r1, pg1[:, 3, :])
                np1 = wk.tile([128, RG], f32, tag=f"np1_{gi}", name="np1")
                V.tensor_add(np1[:], s2[:], pg1[:, 2, :])
                stt["g"], stt["hm"], stt["np1"] = g, hm, np1
                if s < NSTEPS - 1:
                    ntg16 = wk.tile([128, RG], f16, tag=f"ntg16_{gi}",
                                    name="ntg16")
                    V.scalar_tensor_tensor(ntg16[:], g[:], -1.0, h1f[:],
                                           op0=Alu.mult, op1=Alu.mult)
                    z1n = stt["z1ab_next"]
                    mm_seq(z1n[:, 0, 0:RG],
                           [(sb["w0"][:, 0:128], ntg16[:]),
                            (sb["w0_l"][:, 0:128], ntg16[:])],
                           start=False, stop=False)
                    mm_seq(z1n[:, 1, 0:RG],
                           [(sb["w0"][:, 128:256], ntg16[:]),
                            (sb["w0_l"][:, 128:256], ntg16[:])],
                           start=False, stop=False)
                return

            if p == 26:
                n1 = wk.tile([128, RG], f32, tag=f"n1_{gi}", name="n1")
                ACT(n1[:], stt["np1"][:], Act.Tanh)
                stt["n1"] = n1
                return

            if p == 27:
                if s < NSTEPS - 1:
                    vg16 = st.tile([128, RG], f16, tag=f"vg16_{gi}",
                                   name="vg16")
                    V.tensor_mul(vg16[:], stt["g"][:], stt["n1"][:])
                    z1n = stt["z1ab_next"]
                    mm_seq(z1n[:, 0, 0:RG],
                           [(sb["w0"][:, 0:128], vg16[:]),
                            (sb["w0_l"][:, 0:128], vg16[:])],
                           start=False, stop=False)
                    mm_seq(z1n[:, 1, 0:RG],
                           [(sb["w0"][:, 128:256], vg16[:]),
                            (sb["w0_l"][:, 128:256], vg16[:])],
                           start=False, stop=False)
                vg = wk.tile([128, RG], f32, tag=f"vg_{gi}", name="vg")
                V.tensor_mul(vg[:], stt["g"][:], stt["n1"][:])
                hn = st.tile([128, RG], f32, tag=f"hn_{gi}", name="hn")
                V.tensor_add(hn[:], vg[:], stt["hm"][:])
                hprev[gi] = hn
                return


        total = NSTEPS * NP
        EMIT_MODE = os.environ.get("ODERNN_EMIT", "phase")
        if EMIT_MODE == "blocked":
            for s in range(NSTEPS):
                for gi in range(G):
                    for p in range(NP):
                        emit(s, gi, p)
        else:
            for t in range(total + OFF):
                if t < total:
                    emit(t // NP, 0, t % NP)
                u = t - OFF
                if 0 <= u < total:
                    emit(u // NP, 1, u % NP)

        pp.release()
        wk.release()
        st.release()
        wp.release()

    nc.compile()
    return nc


def _f16(x):
    return np.ascontiguousarray(x.astype(F16))


def _prep(inputs):
    x2d = np.asarray(inputs["x2d"], np.float32)
    mask = np.asarray(inputs["mask"])
    g = lambda n: np.asarray(inputs[n], np.float32)
    w0 = g("ode_w0"); w1 = g("ode_w1"); w2 = g("ode_w2")
    wih0, whh0 = g("wih0"), g("whh0")
    wih1, whh1 = g("wih1"), g("whh1")
    wout = g("wout")
    h0 = g("h0")
    for nm in ["ode_b0", "ode_b1", "ode_b2", "bih0", "bhh0", "bih1", "bhh1",
               "bout"]:
        assert not np.any(np.asarray(inputs[nm])), f"nonzero bias {nm}"

    mf = mask.astype(np.float32)
    xs = (x2d * mf).reshape(B, S, N, DIN)[:, :NSTEPS]
    ms = mf.reshape(B, S, N)[:, :NSTEPS]

    W20 = (DT * (w2.astype(np.float64) @ w0.astype(np.float64))).astype(np.float32)
    h0T = np.repeat(h0.reshape(DRNN, 1), R, axis=1).astype(np.float32)

    weights = {
        "w0": w0, "w1a": w1[0:128], "w1b": w1[128:256],
        "w2a": DT * w2[0:128], "w2b": DT * w2[128:256],
        "W20a": W20[0:128], "W20b": W20[128:256],
        "wih0": wih0, "whh0": whh0, "wih1": wih1, "whh1": whh1,
        "wout": wout,
    }
    shared = {"h0f": h0T, "h016": _f16(h0T)}
    for nm in ("W20a", "W20b", "w2a", "w2b"):
        shared[nm + "32"] = np.ascontiguousarray(weights[nm], np.float32)
    for nm, w in weights.items():
        w = np.ascontiguousarray(w, np.float32)
        hi = w.astype(F16)
        shared[nm] = np.ascontiguousarray(hi)
        shared[nm + "_l"] = _f16(w - hi.astype(np.float32))

    in_maps = []
    for c in range(NCORES):
        xc = xs[c * BL:(c + 1) * BL]
        xmT = np.ascontiguousarray(
            xc.transpose(3, 1, 0, 2).reshape(DIN, SR), np.float32)
        xhi = xmT.astype(F16)
        mc = ms[c * BL:(c + 1) * BL]
        mrow = mc.transpose(1, 0, 2).reshape(1, SR)
        mbc = np.broadcast_to(mrow, (DRNN, SR))
        m = dict(shared)
        m["xm"] = np.ascontiguousarray(xhi)
        m["xml"] = _f16(xmT - xhi.astype(np.float32))
        m["mbc"] = np.ascontiguousarray(mbc.astype(BF16))
        in_maps.append(m)
    return in_maps


def kernel(**inputs):
    in_maps = _prep(inputs)
    if "nc" not in _prog_cache:
        _prog_cache["nc"] = _build_program()
    nc = _prog_cache["nc"]

    from concourse.bass_utils import run_bass_kernel_spmd
    res = run_bass_kernel_spmd(nc, in_maps, core_ids=list(range(NCORES)))

    ys = np.zeros((B, NSTEPS, P, J, DOUT), np.float32)
    for c in range(NCORES):
        y = res.results[c]["y"]                      # (DOUT, SR)
        y = y.reshape(DOUT, NSTEPS, BL, N).transpose(2, 1, 3, 0)
        ys[c * BL:(c + 1) * BL] = y.reshape(BL, NSTEPS, P, J, DOUT)
    return ys
